# revision 31
# baseline (speedup 1.0000x reference)
"""AM sign-quantize hamming-similarity kernel for one TRN2 chip (8 NeuronCores).

logit[b, c] = (D + sum_d sign(q[b,d]) * sign(am[c,d])) / 2

Strategy:
  - Data-parallel over batch: each of the 8 cores gets B/8 = 1024 query rows.
  - am (1000 x 10240) is sharded over cores by class rows (125 each); every
    core sign-quantizes + transposes its slice to fp8 and the slices are
    all-gathered (10.2 MB) so each core ends with the full transposed
    sign-AM resident in SBUF: saT [128, 80, 1000] fp8.
  - signs are computed as bits = (x > 0) in {0,1} (one DVE op), shifted to
    {-0.5,+0.5} for free via the bias of the PSUM-evacuation copy on the
    ScalarEngine.  Products +-0.25 accumulate exactly in fp32 PSUM, so the
    matmul is EXACT; logit = 2 * psum + D/2.
  - Both matmul operands need the contraction dim (D) on SBUF partitions:
    transposed on-chip via identity matmuls on the TensorEngine (regular
    matmuls, 4 per PSUM bank, to preserve HAM warmth).
  - Main matmul runs in fp8 DoubleRow perf mode (2 k-tiles per pass).
"""

import sys

if "/opt/trn_rl_repo" not in sys.path:
    sys.path.insert(0, "/opt/trn_rl_repo")

import numpy as np

from concourse import bacc, bass, masks, mybir
from concourse.bass_utils import run_bass_kernel_spmd
from concourse.tile import TileContext

B, D, C = 8192, 10240, 1000
NCORES = 8
BS = B // NCORES  # 1024 batch rows per core
CSH = C // NCORES  # 125 am rows per core

F32 = mybir.dt.float32
BF16 = mybir.dt.bfloat16
FP8 = mybir.dt.float8e4

KT = D // 128  # 80 k-tiles of 128 along the contraction dim

USE_DOUBLE_ROW = True


def build_nc() -> bass.Bass:
    nc = bacc.Bacc(None, target_bir_lowering=False, num_devices=NCORES)
    q_ext = nc.declare_dram_parameter("query", [BS, D], F32, isOutput=False)
    am_ext = nc.declare_dram_parameter("am_weight", [CSH, D], F32, isOutput=False)
    out_ext = nc.declare_dram_parameter("out", [BS, C], mybir.dt.int16, isOutput=True)

    gt = mybir.AluOpType.is_gt
    mult = mybir.AluOpType.mult
    add = mybir.AluOpType.add
    copyf = mybir.ActivationFunctionType.Copy

    with TileContext(nc) as tc:
        with (
            tc.tile_pool(name="const", bufs=1) as constp,
            tc.tile_pool(name="sat", bufs=1) as satp,
            tc.tile_pool(name="dram", bufs=1, space="DRAM") as dramp,
            tc.tile_pool(name="amload", bufs=3) as amload,
            tc.tile_pool(name="amsign", bufs=3) as amsign,
            tc.tile_pool(name="qload", bufs=3) as qload,
            tc.tile_pool(name="qsign", bufs=3) as qsignp,
            tc.tile_pool(name="qt", bufs=2) as qtp,
            tc.tile_pool(name="outp", bufs=3) as outp,
            tc.tile_pool(name="ps_t", bufs=2, space="PSUM") as ps_t,
            tc.tile_pool(name="ps_mm", bufs=2, space="PSUM") as ps_mm,
        ):
            ident = constp.tile([128, 128], BF16)
            masks.make_identity(nc, ident[:])

            # saT[p, k, c] = sign(am[c, k*128 + p]) in {+-0.5}, fp8
            saT = satp.tile([128, KT, C], FP8)
            # this core's transposed slice, and the all-gather bounce buffers
            saTs = satp.tile([128, KT, CSH], FP8)
            KH = KT // 2  # all-gather split in two k-halves for overlap
            b_in0 = dramp.tile([128, KH, CSH], FP8)
            b_in1 = dramp.tile([128, KH, CSH], FP8)
            b_out0 = dramp.tile([NCORES, 128, KH, CSH], FP8, addr_space="Shared")
            b_out1 = dramp.tile([NCORES, 128, KH, CSH], FP8, addr_space="Shared")

            # ----- phase A: slice -> sign -> transpose -> all-gather --------
            DCH_A = 2048
            assert D % DCH_A == 0 and DCH_A % 1024 == 0
            for dch in range(D // DCH_A):
                a_f32 = amload.tile([128, DCH_A], BF16, tag="amload")
                nc.gpsimd.dma_start(
                    out=a_f32[:CSH, :],
                    in_=am_ext[:, dch * DCH_A : (dch + 1) * DCH_A],
                )
                a_bit = amsign.tile([128, DCH_A], BF16, tag="amsign")
                nc.vector.tensor_scalar(a_bit[:CSH, :], a_f32[:CSH, :], 0.0, None, gt)
                for g in range(DCH_A // 1024):
                    pt = ps_t.tile([128, 8, 128], F32, tag="ps_t")  # 2 PSUM banks
                    for j in range(8):
                        kk = g * 8 + j
                        nc.tensor.matmul(
                            pt[:, j, :CSH],
                            a_bit[:CSH, kk * 128 : (kk + 1) * 128],
                            ident[:CSH, :CSH],
                            start=(j % 4 == 0),
                            stop=(j % 4 == 3),
                            skip_group_check=True,
                        )
                    kbase = dch * (DCH_A // 128) + g * 8
                    if (dch * (DCH_A // 1024) + g) % 3 != 2:
                        nc.scalar.activation(
                            saTs[:, kbase : kbase + 8, :],
                            pt[:, :, :CSH],
                            copyf,
                            bias=-0.5,
                        )
                    else:
                        nc.vector.tensor_scalar(
                            saTs[:, kbase : kbase + 8, :],
                            pt[:, :, :CSH],
                            -0.5,
                            None,
                            add,
                        )
            for h, (bi, bo) in enumerate(((b_in0, b_out0), (b_in1, b_out1))):
                nc.sync.dma_start(out=bi[:], in_=saTs[:, h * KH : (h + 1) * KH, :])
                nc.gpsimd.collective_compute(
                    "AllGather",
                    mybir.AluOpType.bypass,
                    replica_groups=[list(range(NCORES))],
                    ins=[bi[:].opt()],
                    outs=[bo[:].opt()],
                )
                for s in range(NCORES):
                    nc.sync.dma_start(
                        out=saT[:, h * KH : (h + 1) * KH, s * CSH : (s + 1) * CSH],
                        in_=bo[s],
                    )

            # ----- phase B: per batch-tile transpose + matmul ---------------
            DCH_B = 2048
            assert D % DCH_B == 0 and DCH_B % 1024 == 0
            for mt in range(8):  # 8 batch tiles of 128 rows
                b0 = mt * 128
                qT = qtp.tile([128, KT, 128], FP8, tag="qt")
                npk = DCH_B // 1024  # 8-deep packs per chunk
                for dch in range(D // DCH_B):
                    qf = qload.tile([128, DCH_B], BF16, tag="qload")
                    nc.gpsimd.dma_start(
                        out=qf[:],
                        in_=q_ext[b0 : b0 + 128, dch * DCH_B : (dch + 1) * DCH_B],
                    )
                    qs = qsignp.tile([128, DCH_B], BF16, tag="qsign")
                    nc.vector.tensor_scalar(qs[:], qf[:], 0.0, None, gt)
                    for g in range(npk):
                        pt = ps_t.tile([128, 8, 128], F32, tag="ps_t")
                        for j in range(8):
                            kk = g * 8 + j
                            nc.tensor.matmul(
                                pt[:, j, :],
                                qs[:, kk * 128 : (kk + 1) * 128],
                                ident[:, :],
                                start=(j % 4 == 0),
                                stop=(j % 4 == 3),
                                skip_group_check=True,
                            )
                        kbase = dch * (8 * npk) + g * 8
                        if (dch * npk + g) % 3 != 2:
                            nc.scalar.activation(
                                qT[:, kbase : kbase + 8, :], pt[:], copyf, bias=-0.5
                            )
                        else:
                            nc.vector.tensor_scalar(
                                qT[:, kbase : kbase + 8, :], pt[:], -0.5, None, add
                            )

                # main matmuls: accumulate over all 80 k-tiles
                pm = ps_mm.tile([128, 1024], F32, tag="ps_mm")
                if USE_DOUBLE_ROW:
                    for kp in range(KT // 2):
                        nc.tensor.matmul(
                            pm[:, 0:512],
                            qT[:, 2 * kp : 2 * kp + 2, :],
                            saT[:, 2 * kp : 2 * kp + 2, 0:512],
                            start=(kp == 0),
                            stop=(kp == KT // 2 - 1),
                            perf_mode=mybir.MatmulPerfMode.DoubleRow,
                        )
                        nc.tensor.matmul(
                            pm[:, 512:1000],
                            qT[:, 2 * kp : 2 * kp + 2, :],
                            saT[:, 2 * kp : 2 * kp + 2, 512:1000],
                            start=(kp == 0),
                            stop=(kp == KT // 2 - 1),
                            perf_mode=mybir.MatmulPerfMode.DoubleRow,
                        )
                else:
                    for k in range(KT):
                        nc.tensor.matmul(
                            pm[:, 0:512],
                            qT[:, k, :],
                            saT[:, k, 0:512],
                            start=(k == 0),
                            stop=(k == KT - 1),
                        )
                        nc.tensor.matmul(
                            pm[:, 512:1000],
                            qT[:, k, :],
                            saT[:, k, 512:1000],
                            start=(k == 0),
                            stop=(k == KT - 1),
                        )

                # logit = 2 * psum + D/2 (an exact even-parity integer <= 10240)
                ot = outp.tile([128, C], mybir.dt.int16, tag="outp")
                nc.scalar.activation(ot[:], pm[:, 0:C], copyf, bias=float(D) / 2, scale=2.0)
                nc.sync.dma_start(out=out_ext[b0 : b0 + 128, :], in_=ot[:])

    nc.compile()
    return nc


_NC = None


def kernel(query: np.ndarray, am_weight: np.ndarray) -> np.ndarray:
    global _NC
    if _NC is None:
        _NC = build_nc()
    query = np.ascontiguousarray(query, dtype=np.float32)
    am_weight = np.ascontiguousarray(am_weight, dtype=np.float32)
    in_maps = [
        {
            "query": query[i * BS : (i + 1) * BS],
            "am_weight": am_weight[i * CSH : (i + 1) * CSH],
        }
        for i in range(NCORES)
    ]
    res = run_bass_kernel_spmd(_NC, in_maps, core_ids=list(range(NCORES)))
    return np.concatenate(
        [res.results[i]["out"].astype(np.float32) for i in range(NCORES)], axis=0
    )


if __name__ == "__main__":
    q = np.random.randn(B, D).astype(np.float32)
    a = np.random.randn(C, D).astype(np.float32)
    out = kernel(q, a)
    sq = np.where(q > 0, 1.0, -1.0).astype(np.float32)
    sa = np.where(a > 0, 1.0, -1.0).astype(np.float32)
    ref = (D + sq @ sa.T) * 0.5
    err = np.abs(out - ref).max()
    print("max abs err:", err)
